# revision 17
# baseline (speedup 1.0000x reference)
"""Associative-embedding loss kernel for 8 Trainium2 NeuronCores.

Math: per image b, with tl[n,c] = pred[b,c,ty,tx] and br[n,c] = target[b,c,by,bx]
gathered at the N=128 match points:
  pull_b = sum_{n,c} (tl-br)^2 / (2N)
  s[n]   = sum_c (tl+br),  A'[i,j] = s[i]-s[j]   (A = A'/2)
  push_b = (0.5*(sum|A'+2| - sum|A'|) - N) / (N(N-1))
using sum_{ij} relu(1-|A|) = sum|A'+2| - sum|A'| for antisymmetric A'
(the diagonal contributes 2N, removed on the host).

Strategy: data-parallel over B (8 images per core); the host does the
point gather (HW indirect DMA is ~1.3us per index row, so extraction is
host-side) and uploads ONLY the gathered points — every loss FLOP runs
on device.  ~18 instructions, two parallel ~31KB fp8 uploads (40
per-partition descriptors each — multiples of 8 partitions matter: a
41-partition transfer trickles ~1.5us slower, and one merged 61KB
transfer on a single HWDGE queue is ~2.8us slower) and one 24B result
DMA.

The core trick: A'[i, 128b+j] = s_b[i] - s_b[j] for 4 images is ONE
K=40 matmul per 512-column PSUM bank — no on-device assembly at all.
Each upload half carries exactly what one bank contracts:
  lhsT W [40,128]: rows 0:16 tl[b,:,c] (row 4b+c), rows 16:32 br,
      rows 32:40 = -1                        (all uploaded)
  rhs [40,512]: rows 0:32 block-diagonal indicator (row 4b+c is 1 on
      column block b), rows 32:40 the same values in flat layout
      (row q = corner/channel q, column 128b+j = point j of image b)
  out[i, 128b+j] = sum_c tl[b,i,c] + sum_c br[b,i,c]     (indicator)
                   - sum_q v[q, 128b+j]                   (-1 rows)
                 = s_b[i] - s_b[j]
i.e. the per-point channel sums s are computed inline by the
contraction itself.  ScalarE row-reduces |A'+2| (Abs with bias 2 via
accum_out, main out to a junk PSUM bank — ScE writes PSUM faster than
SBUF), VectorE row-reduces |A'|, ping-ponged across banks so the
engines never contend for a PSUM bank read port; pull is a DVE
subtract + square-accumulate per half.  The six per-core partials fold
to one [1,6] f32 row via two ones-vector matmuls (partition reduction
+ transpose in one PE op) so the result DMA is a single descriptor.
fp8e4 uploads only perturb the result ~8e-4, far inside the 2e-2 gate
(the indicator and -1 constants are exact in fp8).
"""

import numpy as np

B, C, H, W, N = 64, 4, 256, 256, 128
M = 8            # cores
BL = B // M      # images per core
HC = 512 + 2 * N  # columns per upload half

_GRAPH = None

_INDH = np.repeat(np.kron(np.eye(4, dtype=np.float32),
                          np.ones((1, N), np.float32)), 4, axis=0)  # [16, 512]
_MB = np.zeros((1, HC), np.float32)
_MB[0, 0:512] = 1.0        # rhs side of the +2 row
_MB[0, 512:512 + N] = 2.0  # lhsT side
import ml_dtypes as _mld
_MB = _MB.astype(_mld.float8_e4m3)


def _build_graph():
    import concourse.bacc as bacc
    import concourse.mybir as mybir
    from concourse.tile import TileContext

    f32 = mybir.dt.float32
    fp8 = mybir.dt.float8e4
    Alu = mybir.AluOpType
    Act = mybir.ActivationFunctionType
    Axis = mybir.AxisListType

    nc = bacc.Bacc()
    m1_d = nc.declare_dram_parameter("m1", [40, HC], fp8, isOutput=False)
    m2_d = nc.declare_dram_parameter("m2", [40, HC], fp8, isOutput=False)
    mb_d = nc.declare_dram_parameter("mb", [1, HC], fp8, isOutput=False)
    o_d = nc.declare_dram_parameter("o", [1, 8], f32, isOutput=True)

    with TileContext(nc) as tc:
        with (
            tc.tile_pool(name="sb", bufs=1) as pool,
            tc.tile_pool(name="ps", bufs=1, space="PSUM") as psum,
        ):
            mg = pool.tile([40, HC], fp8)
            nc.sync.dma_start(out=mg[:], in_=m1_d[:])
            mh = pool.tile([48, HC], fp8)
            nc.scalar.dma_start(out=mh[0:40, :], in_=m2_d[:])
            # K=41 bias row (lhsT=2, rhs=1 -> A'+2 bank): one descriptor
            nc.sync.dma_start(out=mh[40:41, :], in_=mb_d[:])

            ones = pool.tile([128, 1], f32)
            nc.vector.memset(ones[:], 1.0)
            two = pool.tile([128, 1], f32)
            nc.vector.memset(two[:], 2.0)
            acc = pool.tile([128, 4], f32)   # pullA, pullB, |A'| A, |A'| B
            nc.vector.memset(acc[:], 0.0)
            accs = pool.tile([128, 2], f32)  # |A'+2| per bank (scalar engine)

            dA = pool.tile([16, N], f32)
            nc.vector.tensor_sub(dA[:], mg[0:16, 512:512 + N],
                                 mg[0:16, 512 + N:HC])
            d2A = pool.tile([16, N], f32)
            nc.vector.scalar_tensor_tensor(
                out=d2A[:], in0=dA[:], scalar=0.0, in1=dA[:],
                op0=Alu.bypass, op1=Alu.mult, accum_out=acc[0:16, 0:1])
            dB = pool.tile([16, N], f32)
            nc.vector.tensor_sub(dB[:], mh[0:16, 512:512 + N],
                                 mh[0:16, 512 + N:HC])
            d2B = pool.tile([16, N], f32)
            nc.vector.scalar_tensor_tensor(
                out=d2B[:], in0=dB[:], scalar=0.0, in1=dB[:],
                op0=Alu.bypass, op1=Alu.mult, accum_out=acc[0:16, 1:2])

            bankA = psum.tile([128, 512], f32, name="bankA", tag="a")
            bankB = psum.tile([128, 512], f32, name="bankB", tag="b")
            bankB2 = psum.tile([128, 512], f32, name="bankB2", tag="b2")
            nc.tensor.matmul(out=bankA[:], lhsT=mg[:, 512:512 + N],
                             rhs=mg[:, 0:512], start=True, stop=True)
            nc.tensor.matmul(out=bankB[:], lhsT=mh[0:40, 512:512 + N],
                             rhs=mh[0:40, 0:512], start=True, stop=True)
            nc.tensor.matmul(out=bankB2[:], lhsT=mh[0:41, 512:512 + N],
                             rhs=mh[0:41, 0:512], start=True, stop=True)

            # one reducer engine per bank — no cross-engine PSUM handoffs
            scr = psum.tile([128, 512], f32, name="scr", tag="scr")
            nc.scalar.activation(
                out=scr[:], in_=bankA[:], func=Act.Abs,
                bias=two[:, 0:1], scale=1.0, accum_out=accs[:, 0:1])
            nc.scalar.activation(
                out=scr[:], in_=bankA[:], func=Act.Abs,
                bias=0.0, scale=1.0, accum_out=accs[:, 1:2])
            nc.vector.tensor_reduce(
                out=acc[:, 2:3], in_=bankB[:], axis=Axis.X,
                op=Alu.add, apply_absolute_value=True)
            nc.vector.tensor_reduce(
                out=acc[:, 3:4], in_=bankB2[:], axis=Axis.X,
                op=Alu.add, apply_absolute_value=True)

            pr = psum.tile([1, 8], f32, name="pr", tag="pr")
            nc.tensor.matmul(out=pr[0:1, 4:6], lhsT=ones[:], rhs=accs[:],
                             start=True, stop=True)
            nc.tensor.matmul(out=pr[0:1, 0:4], lhsT=ones[:], rhs=acc[:],
                             start=True, stop=True)
            res = pool.tile([1, 8], f32)
            nc.vector.tensor_copy(res[:], pr[:])
            nc.sync.dma_start(out=o_d[:], in_=res[:])
    nc.finalize()
    return nc


def _get_graph():
    global _GRAPH
    if _GRAPH is None:
        _GRAPH = _build_graph()
    return _GRAPH


def _half(tls, brs):
    K4 = 4 * N
    m = np.zeros((40, HC), np.float32)
    m[0:16, 0:512] = _INDH
    m[16:32, 0:512] = _INDH
    m[32:36, 0:512] = tls.transpose(2, 0, 1).reshape(4, K4)
    m[36:40, 0:512] = brs.transpose(2, 0, 1).reshape(4, K4)
    m[0:16, 512:512 + N] = tls.transpose(0, 2, 1).reshape(16, N)
    m[16:32, 512:512 + N] = brs.transpose(0, 2, 1).reshape(16, N)
    m[32:40, 512:512 + N] = -1.0
    m[0:16, 512 + N:HC] = brs.transpose(0, 2, 1).reshape(16, N)
    return m


def _make_in_maps(pred, target, match):
    import ml_dtypes

    fp8 = ml_dtypes.float8_e4m3
    barr = np.arange(B)[:, None]
    tl = pred[barr, :, match[:, :, 0, 0], match[:, :, 0, 1]]    # [B, N, C]
    br = target[barr, :, match[:, :, 1, 0], match[:, :, 1, 1]]  # [B, N, C]

    in_maps = []
    for i in range(M):
        s0 = i * BL
        m1 = _half(tl[s0:s0 + 4], br[s0:s0 + 4])
        m2 = _half(tl[s0 + 4:s0 + 8], br[s0 + 4:s0 + 8])
        in_maps.append({"m1": m1.astype(fp8), "m2": m2.astype(fp8),
                        "mb": _MB})
    return in_maps


def _finish(core_outs):
    pull_total = 0.0
    m_total = 0.0
    for o in core_outs:
        o = np.asarray(o, dtype=np.float64)
        pull_total += o[0, 0] + o[0, 1]
        m_total += o[0, 3] + o[0, 4] - o[0, 2] - o[0, 5]
    pull_all = 0.25 * pull_total / (2 * N)
    push_all = 0.25 * (0.5 * m_total - B * N) / (N * (N - 1))
    return (np.float32(pull_all), np.float32(push_all))


def kernel(pred, target, match):
    from concourse.bass_utils import run_bass_kernel_spmd

    nc = _get_graph()
    in_maps = _make_in_maps(np.asarray(pred), np.asarray(target), np.asarray(match))
    res = run_bass_kernel_spmd(nc, in_maps, core_ids=list(range(M)))
    return _finish([r["o"] for r in res.results])


# revision 18
# speedup vs baseline: 1.0191x; 1.0191x over previous
"""Associative-embedding loss kernel for 8 Trainium2 NeuronCores.

Math: per image b, with tl[n,c] = pred[b,c,ty,tx] and br[n,c] = target[b,c,by,bx]
gathered at the N=128 match points:
  pull_b = sum_{n,c} (tl-br)^2 / (2N)
  s[n]   = sum_c (tl+br),  A'[i,j] = s[i]-s[j]   (A = A'/2)
  push_b = (0.5*(sum|A'+2| - sum|A'|) - N) / (N(N-1))
using sum_{ij} relu(1-|A|) = sum|A'+2| - sum|A'| for antisymmetric A'
(the diagonal contributes 2N, removed on the host).

Strategy: data-parallel over B (8 images per core); the host does the
point gather (HW indirect DMA is ~1.3us per index row, so extraction is
host-side) and uploads ONLY the gathered points — every loss FLOP runs
on device.  ~19 instructions, two parallel ~31KB fp8 uploads (40
per-partition descriptors each — multiples of 8 partitions matter: a
41- or 48-partition transfer straggles ~1.5us; one merged 61KB
transfer on a single HWDGE queue is ~2.8us slower), a one-descriptor
bias-row upload, and one 24B result DMA.

The core trick: A'[i, 128b+j] = s_b[i] - s_b[j] for 4 images is ONE
K=40 matmul per 512-column PSUM bank — no on-device assembly at all.
A third bank holds A'+2 for the second half via a K=41 matmul over the
same columns: row 40 (delivered by the single-descriptor DMA so the
main uploads keep the fast 40-partition shape) is lhsT=2 / rhs=1,
adding +2 to every element inline.  Each upload half carries exactly
what one bank contracts:
  lhsT W [40,128]: rows 0:16 tl[b,:,c] (row 4b+c), rows 16:32 br,
      rows 32:40 = -1                        (all uploaded)
  rhs [40,512]: rows 0:32 block-diagonal indicator (row 4b+c is 1 on
      column block b), rows 32:40 the same values in flat layout
      (row q = corner/channel q, column 128b+j = point j of image b)
  out[i, 128b+j] = sum_c tl[b,i,c] + sum_c br[b,i,c]     (indicator)
                   - sum_q v[q, 128b+j]                   (-1 rows)
                 = s_b[i] - s_b[j]
i.e. the per-point channel sums s are computed inline by the
contraction itself.  Each reducer engine owns one half outright (no
cross-engine PSUM bank handoffs): ScalarE row-reduces |A'+2| and |A'|
on bank A (Abs with bias 2 / bias 0 via accum_out, main out to a junk
PSUM bank — ScE writes PSUM faster than SBUF), VectorE abs-reduces
bank B and the A'+2 bank; pull is a DVE subtract + square-accumulate
per half.  The six per-core partials fold to one [1,6] f32 row via two
ones-vector matmuls (partition reduction + transpose in one PE op) so
the result DMA is a single descriptor.
fp8e4 uploads only perturb the result ~8e-4, far inside the 2e-2 gate
(the indicator and -1 constants are exact in fp8).
"""

import numpy as np

B, C, H, W, N = 64, 4, 256, 256, 128
M = 8            # cores
BL = B // M      # images per core
HC = 512 + 2 * N  # columns per upload half

_GRAPH = None

_INDH = np.repeat(np.kron(np.eye(4, dtype=np.float32),
                          np.ones((1, N), np.float32)), 4, axis=0)  # [16, 512]
_MB = np.zeros((1, HC), np.float32)
_MB[0, 0:512] = 1.0        # rhs side of the +2 row
_MB[0, 512:512 + N] = 2.0  # lhsT side
import ml_dtypes as _mld
_MB = _MB.astype(_mld.float8_e4m3)


def _build_graph():
    import concourse.bacc as bacc
    import concourse.mybir as mybir
    from concourse.tile import TileContext

    f32 = mybir.dt.float32
    fp8 = mybir.dt.float8e4
    Alu = mybir.AluOpType
    Act = mybir.ActivationFunctionType
    Axis = mybir.AxisListType

    nc = bacc.Bacc()
    m1_d = nc.declare_dram_parameter("m1", [40, HC], fp8, isOutput=False)
    m2_d = nc.declare_dram_parameter("m2", [40, HC], fp8, isOutput=False)
    mb_d = nc.declare_dram_parameter("mb", [1, HC], fp8, isOutput=False)
    o_d = nc.declare_dram_parameter("o", [1, 8], f32, isOutput=True)

    with TileContext(nc) as tc:
        with (
            tc.tile_pool(name="sb", bufs=1) as pool,
            tc.tile_pool(name="ps", bufs=1, space="PSUM") as psum,
        ):
            mg = pool.tile([40, HC], fp8)
            nc.sync.dma_start(out=mg[:], in_=m1_d[:])
            mh = pool.tile([48, HC], fp8)
            nc.scalar.dma_start(out=mh[0:40, :], in_=m2_d[:])
            # K=41 bias row (lhsT=2, rhs=1 -> A'+2 bank): one descriptor
            nc.sync.dma_start(out=mh[40:41, :], in_=mb_d[:])

            ones = pool.tile([128, 1], f32)
            nc.vector.memset(ones[:], 1.0)
            two = pool.tile([128, 1], f32)
            nc.vector.memset(two[:], 2.0)
            acc = pool.tile([128, 4], f32)   # pullA, pullB, |A'| A, |A'| B
            nc.vector.memset(acc[:], 0.0)
            accs = pool.tile([128, 2], f32)  # |A'+2| per bank (scalar engine)

            dA = pool.tile([16, N], f32)
            nc.vector.tensor_sub(dA[:], mg[0:16, 512:512 + N],
                                 mg[0:16, 512 + N:HC])
            d2A = pool.tile([16, N], f32)
            nc.vector.scalar_tensor_tensor(
                out=d2A[:], in0=dA[:], scalar=0.0, in1=dA[:],
                op0=Alu.bypass, op1=Alu.mult, accum_out=acc[0:16, 0:1])
            dB = pool.tile([16, N], f32)
            nc.vector.tensor_sub(dB[:], mh[0:16, 512:512 + N],
                                 mh[0:16, 512 + N:HC])
            d2B = pool.tile([16, N], f32)
            nc.vector.scalar_tensor_tensor(
                out=d2B[:], in0=dB[:], scalar=0.0, in1=dB[:],
                op0=Alu.bypass, op1=Alu.mult, accum_out=acc[0:16, 1:2])

            bankA = psum.tile([128, 512], f32, name="bankA", tag="a")
            bankB = psum.tile([128, 512], f32, name="bankB", tag="b")
            bankB2 = psum.tile([128, 512], f32, name="bankB2", tag="b2")
            nc.tensor.matmul(out=bankA[:], lhsT=mg[:, 512:512 + N],
                             rhs=mg[:, 0:512], start=True, stop=True)
            nc.tensor.matmul(out=bankB[:], lhsT=mh[0:40, 512:512 + N],
                             rhs=mh[0:40, 0:512], start=True, stop=True)
            nc.tensor.matmul(out=bankB2[:], lhsT=mh[0:41, 512:512 + N],
                             rhs=mh[0:41, 0:512], start=True, stop=True)

            # one reducer engine per bank — no cross-engine PSUM handoffs
            scr = psum.tile([128, 512], f32, name="scr", tag="scr")
            nc.scalar.activation(
                out=scr[:], in_=bankA[:], func=Act.Abs,
                bias=two[:, 0:1], scale=1.0, accum_out=accs[:, 0:1])
            nc.scalar.activation(
                out=scr[:], in_=bankA[:], func=Act.Abs,
                bias=0.0, scale=1.0, accum_out=accs[:, 1:2])
            nc.vector.tensor_reduce(
                out=acc[:, 2:3], in_=bankB[:], axis=Axis.X,
                op=Alu.add, apply_absolute_value=True)
            nc.vector.tensor_reduce(
                out=acc[:, 3:4], in_=bankB2[:], axis=Axis.X,
                op=Alu.add, apply_absolute_value=True)

            pr = psum.tile([1, 8], f32, name="pr", tag="pr")
            nc.tensor.matmul(out=pr[0:1, 4:6], lhsT=ones[:], rhs=accs[:],
                             start=True, stop=True)
            nc.tensor.matmul(out=pr[0:1, 0:4], lhsT=ones[:], rhs=acc[:],
                             start=True, stop=True)
            res = pool.tile([1, 8], f32)
            nc.vector.tensor_copy(res[:], pr[:])
            nc.sync.dma_start(out=o_d[:], in_=res[:])
    nc.finalize()
    return nc


def _get_graph():
    global _GRAPH
    if _GRAPH is None:
        _GRAPH = _build_graph()
    return _GRAPH


def _half(tls, brs):
    K4 = 4 * N
    m = np.zeros((40, HC), np.float32)
    m[0:16, 0:512] = _INDH
    m[16:32, 0:512] = _INDH
    m[32:36, 0:512] = tls.transpose(2, 0, 1).reshape(4, K4)
    m[36:40, 0:512] = brs.transpose(2, 0, 1).reshape(4, K4)
    m[0:16, 512:512 + N] = tls.transpose(0, 2, 1).reshape(16, N)
    m[16:32, 512:512 + N] = brs.transpose(0, 2, 1).reshape(16, N)
    m[32:40, 512:512 + N] = -1.0
    m[0:16, 512 + N:HC] = brs.transpose(0, 2, 1).reshape(16, N)
    return m


def _make_in_maps(pred, target, match):
    import ml_dtypes

    fp8 = ml_dtypes.float8_e4m3
    barr = np.arange(B)[:, None]
    tl = pred[barr, :, match[:, :, 0, 0], match[:, :, 0, 1]]    # [B, N, C]
    br = target[barr, :, match[:, :, 1, 0], match[:, :, 1, 1]]  # [B, N, C]

    in_maps = []
    for i in range(M):
        s0 = i * BL
        m1 = _half(tl[s0:s0 + 4], br[s0:s0 + 4])
        m2 = _half(tl[s0 + 4:s0 + 8], br[s0 + 4:s0 + 8])
        in_maps.append({"m1": m1.astype(fp8), "m2": m2.astype(fp8),
                        "mb": _MB})
    return in_maps


def _finish(core_outs):
    pull_total = 0.0
    m_total = 0.0
    for o in core_outs:
        o = np.asarray(o, dtype=np.float64)
        pull_total += o[0, 0] + o[0, 1]
        m_total += o[0, 3] + o[0, 4] - o[0, 2] - o[0, 5]
    pull_all = 0.25 * pull_total / (2 * N)
    push_all = 0.25 * (0.5 * m_total - B * N) / (N * (N - 1))
    return (np.float32(pull_all), np.float32(push_all))


def kernel(pred, target, match):
    from concourse.bass_utils import run_bass_kernel_spmd

    nc = _get_graph()
    in_maps = _make_in_maps(np.asarray(pred), np.asarray(target), np.asarray(match))
    res = run_bass_kernel_spmd(nc, in_maps, core_ids=list(range(M)))
    return _finish([r["o"] for r in res.results])
